# revision 35
# baseline (speedup 1.0000x reference)
"""Trainium2 Bass kernel for nn_NegF1: distributed -F1 loss over 16.7M elements.

Data-parallel over 8 NeuronCores; each core streams its 2,097,152-element
slice of probs (f32) / lbls (int32) from HBM.

Per [128, F] tile, DVE writes three bf16 planes into one interleaved
"comb" buffer laid out [128][chunk c][slot s][128] (chunk = 128 columns):
  slot 0: pb = bf16(p)
  slot 1: y  = g * pb   (masked probs; exact mask, bf16-rounded magnitude)
  slot 2: g  = is_gt(p, 0.5)   (computed on fp32 p -> bit-exact vs reference)
ACT casts lb = bf16(l) with fused accum -> Npos, and runs a Copy pass over
y with fused accum -> Y = sum(y).

The TensorEngine does the l-masked reductions via the diagonal trick:
  lhsT = lb chunk [128,128], rhs = comb chunk [128, 3*128], accumulated
  into one PSUM tile [128, 384]; diag of block s over chunks gives
  Sx = sum(l*pb), TP = sum(l*y), C = sum(l*g).

Host combine (float64):
  FP = Y - TP;  FN = Npos - C - Sx + TP
  f1 from TP/FP/FN with eps=1e-5;  return -f1 as float32 scalar.
"""

from contextlib import ExitStack

import numpy as np

N_TOTAL = 16777216
N_CORES = 8
M_PER_CORE = N_TOTAL // N_CORES   # 2097152
P = 128                           # SBUF partitions
EPS = 1e-05
CH = 128                          # diag chunk columns

_CACHE = {}


def build_nc(M=M_PER_CORE, F=2048, bufs=3, in_bufs=4, warmup_mms=12,
             dual_queue=True, debug=False):
    import concourse.bacc as bacc
    import concourse.mybir as mybir
    import concourse.tile as tile

    assert M % (P * F) == 0 and F % CH == 0
    # Taper the first/last tiles: shorter pipeline fill at the start and a
    # shorter compute drain after the last input DMA.
    taper = [F // 4] * 4
    body = (M // P - 2 * F) // F
    tiles = taper + [F] * body + taper
    assert sum(tiles) == M // P
    T = len(tiles)

    f32 = mybir.dt.float32
    i32 = mybir.dt.int32
    bf16 = mybir.dt.bfloat16
    Alu = mybir.AluOpType
    Act = mybir.ActivationFunctionType

    nc = bacc.Bacc("TRN2", target_bir_lowering=False, debug=debug,
                   num_devices=N_CORES)

    probs = nc.dram_tensor("probs", [M], f32, kind="ExternalInput")
    lbls = nc.dram_tensor("lbls", [M], i32, kind="ExternalInput")
    out_diag = nc.dram_tensor("out_diag", [2, P, 3 * CH], f32,
                              kind="ExternalOutput")
    out_ones = nc.dram_tensor("out_ones", [1, 512], f32,
                              kind="ExternalOutput")
    out_acc = nc.dram_tensor("out_acc", [P, T], f32,
                             kind="ExternalOutput")

    # per-tile DRAM views: tile t covers columns [off, off+Ft) of [P, M//P]
    p2 = probs.ap().rearrange("(p f) -> p f", p=P, f=M // P)
    l2 = lbls.ap().rearrange("(p f) -> p f", p=P, f=M // P)

    with tile.TileContext(nc) as tc, ExitStack() as ctx:
        pin = ctx.enter_context(tc.tile_pool(name="pin", bufs=in_bufs))
        lin = ctx.enter_context(tc.tile_pool(name="lin", bufs=in_bufs))
        lbpool = ctx.enter_context(tc.tile_pool(name="lbpool", bufs=bufs))
        cpool = ctx.enter_context(tc.tile_pool(name="cpool", bufs=bufs))
        jpool = ctx.enter_context(tc.tile_pool(name="jpool", bufs=2))
        accp = ctx.enter_context(tc.tile_pool(name="accp", bufs=1))
        psump = ctx.enter_context(tc.tile_pool(name="psump", bufs=1,
                                               space="PSUM"))

        # acc columns 0:T = Npos partials
        acc = accp.tile([P, T], f32)
        ones = accp.tile([P, 1], bf16)
        nc.vector.memset(ones[:], 1.0)

        # two alternating diag accumulators (even / odd chunks) so
        # back-to-back accumulating matmuls don't chain on one PSUM bank
        ps_diag0 = psump.tile([P, 3 * CH], f32)
        ps_diag1 = psump.tile([P, 3 * CH], f32)
        ps_y = psump.tile([1, 512], f32)
        # zero-fill so unwritten tail columns (sub-512 ones-MMs) read as 0
        nc.vector.memset(ps_y[:], 0.0)

        # Warm the PE HAM clock-gate (1.2 -> 2.4 GHz needs ~3.4us sustained)
        # while the first input DMAs are in flight.
        if warmup_mms:
            wu = accp.tile([P, 3 * CH], bf16)
            nc.vector.memset(wu[:], 0.0)
            ps_wu = psump.tile([P, 3 * CH], f32)
            for i in range(warmup_mms):
                nc.tensor.matmul(ps_wu[:, :], wu[:, :CH], wu[:],
                                 start=(i == 0), stop=(i == warmup_mms - 1))

        nctot = sum(tiles) // CH        # total diag chunks
        # Bank routing: alternate even/odd, but the final tile's chunks all
        # go to bank1 so bank0's PSUM->SBUF copy overlaps the last tile.
        lastnc = tiles[-1] // CH
        bank = [1 if i >= nctot - lastnc else i % 2 for i in range(nctot)]
        b0_stop = max(i for i, b in enumerate(bank) if b == 0)
        b1_stop = max(i for i, b in enumerate(bank) if b == 1)
        ymms = sum(-(-(Ft // CH) // 4) for Ft in tiles)  # total ones-MMs
        ci = 0
        yi = 0
        off = 0
        diag_sb = accp.tile([P, 2 * 3 * CH], f32)
        for t, Ft in enumerate(tiles):
            NCt = Ft // CH
            sl = slice(off, off + Ft)
            off += Ft

            pt = pin.tile([P, F], f32, tag="pt")
            nc.sync.dma_start(out=pt[:, :Ft], in_=p2[:, sl])
            lt = lin.tile([P, F], i32, tag="lt")
            nc.sync.dma_start(out=lt[:, :Ft], in_=l2[:, sl])

            # lb = bf16(l); fused accum -> Npos partials
            lb = lbpool.tile([P, F], bf16, tag="lb")
            nc.scalar.activation(lb[:, :Ft], lt[:, :Ft], Act.Copy,
                                 accum_out=acc[:, t:t + 1])

            comb = cpool.tile([P, 3 * F], bf16, tag="comb")
            c4 = comb[:].rearrange("p (c s j) -> p c s j", c=F // CH, s=3,
                                   j=CH)[:, :NCt]
            pt4 = pt[:, :Ft].rearrange("p (c j) -> p c j", c=NCt, j=CH)

            # slot 0: pb = bf16(p)
            nc.vector.tensor_copy(out=c4[:, :, 0, :], in_=pt4)
            # slot 2: g = [p > 0.5]  (fp32 compare)
            nc.vector.tensor_scalar(out=c4[:, :, 2, :], in0=pt4,
                                    scalar1=0.5, scalar2=None, op0=Alu.is_gt)
            # slot 1: y = g * pb
            nc.vector.tensor_mul(out=c4[:, :, 1, :], in0=c4[:, :, 2, :],
                                 in1=c4[:, :, 0, :])

            # diag reductions: ps_diag{0,1} += lb_c.T @ comb_c
            for c in range(NCt):
                ps = ps_diag0 if bank[ci] == 0 else ps_diag1
                nc.tensor.matmul(
                    ps[:, :], lb[:, c * CH:(c + 1) * CH],
                    c4[:, c, :, :],
                    start=(ci in (0, 1)),
                    stop=(ci in (b0_stop, b1_stop)))
                ci += 1
            if t == T - 2:
                # bank0 is complete; drain it while the last tile runs
                nc.scalar.copy(diag_sb[:, :3 * CH], ps_diag0[:, :])

            # Y = sum(y) via ones-matmuls (<=4 chunks of the y plane per MM)
            for c in range(0, NCt, 4):
                hi = min(c + 4, NCt)
                nc.tensor.matmul(
                    ps_y[0:1, :(hi - c) * CH], ones[:], c4[:, c:hi, 1, :],
                    start=(yi == 0), stop=(yi == ymms - 1))
                yi += 1

        # PSUM -> SBUF -> DRAM (bank1 on ACT, ones on DVE: runs in parallel)
        nc.scalar.copy(diag_sb[:, 3 * CH:], ps_diag1[:, :])
        ones_sb = accp.tile([1, 512], f32)
        nc.vector.tensor_copy(out=ones_sb[:], in_=ps_y[0:1, :])

        d3 = out_diag.ap().rearrange("b p c -> p b c")
        sb3 = diag_sb[:].rearrange("p (b c) -> p b c", b=2, c=3 * CH)
        nc.sync.dma_start(out=d3, in_=sb3)
        nc.sync.dma_start(out=out_ones.ap(), in_=ones_sb[:])
        nc.sync.dma_start(out=out_acc.ap(), in_=acc[:])

    nc.compile()
    return nc, T


def get_nc():
    if "nc" not in _CACHE:
        _CACHE["nc"] = build_nc()
    return _CACHE["nc"]


def run_device(probs, lbls, trace=False, **run_kwargs):
    """Run the SPMD kernel; returns (per-core result dicts, BassKernelResults)."""
    from concourse import bass_utils

    nc, _ = get_nc()
    probs = np.ascontiguousarray(probs, dtype=np.float32)
    lbls = np.ascontiguousarray(lbls, dtype=np.int32)
    assert probs.shape == (N_TOTAL,) and lbls.shape == (N_TOTAL,)
    M = M_PER_CORE
    in_maps = [
        {"probs": probs[c * M:(c + 1) * M], "lbls": lbls[c * M:(c + 1) * M]}
        for c in range(N_CORES)
    ]
    res = bass_utils.run_bass_kernel_spmd(
        nc, in_maps, core_ids=list(range(N_CORES)), trace=trace, **run_kwargs)
    return res.results, res


def combine(results):
    """Combine per-core partial sums into the final -f1 scalar."""
    Sx = TP = C = Y = Npos = 0.0
    for r in results:
        dg = np.asarray(r["out_diag"], dtype=np.float64).reshape(2, P, 3, CH)
        for b in range(2):
            Sx += np.trace(dg[b, :, 0, :])
            TP += np.trace(dg[b, :, 1, :])
            C += np.trace(dg[b, :, 2, :])
        Y += np.asarray(r["out_ones"], dtype=np.float64).sum()
        Npos += np.asarray(r["out_acc"], dtype=np.float64).sum()

    FP = Y - TP
    FN = Npos - C - Sx + TP
    precision = (TP + EPS) / (TP + FP + EPS)
    recall = (TP + EPS) / (TP + FN + EPS)
    f1 = 2.0 * precision * recall / (precision + recall)
    return np.float32(-f1)


def kernel(probs, lbls):
    results, _ = run_device(probs, lbls)
    return np.asarray(combine(results), dtype=np.float32)


if __name__ == "__main__":
    rng = np.random.default_rng(0)
    p = rng.uniform(0, 1, N_TOTAL).astype(np.float32)
    l = rng.integers(0, 2, N_TOTAL).astype(np.int32)
    out = kernel(p, l)
    print("kernel output:", out)


# revision 36
# speedup vs baseline: 1.0788x; 1.0788x over previous
"""Trainium2 Bass kernel for nn_NegF1: distributed -F1 loss over 16.7M elements.

Data-parallel over 8 NeuronCores; each core streams its 2,097,152-element
slice of probs (f32) / lbls (int32) from HBM.

Per [128, F] tile, DVE writes three bf16 planes into one interleaved
"comb" buffer laid out [128][chunk c][slot s][128] (chunk = 128 columns):
  slot 0: pb = bf16(p)
  slot 1: y  = g * pb   (masked probs; exact mask, bf16-rounded magnitude)
  slot 2: g  = is_gt(p, 0.5)   (computed on fp32 p -> bit-exact vs reference)
ACT casts lb = bf16(l) with fused accum -> Npos, and runs a Copy pass over
y with fused accum -> Y = sum(y).

The TensorEngine does the l-masked reductions via the diagonal trick:
  lhsT = lb chunk [128,128], rhs = comb chunk [128, 3*128], accumulated
  into one PSUM tile [128, 384]; diag of block s over chunks gives
  Sx = sum(l*pb), TP = sum(l*y), C = sum(l*g).

Host combine (float64):
  FP = Y - TP;  FN = Npos - C - Sx + TP
  f1 from TP/FP/FN with eps=1e-5;  return -f1 as float32 scalar.
"""

from contextlib import ExitStack

import numpy as np

N_TOTAL = 16777216
N_CORES = 8
M_PER_CORE = N_TOTAL // N_CORES   # 2097152
P = 128                           # SBUF partitions
EPS = 1e-05
CH = 128                          # diag chunk columns

_CACHE = {}


def build_nc(M=M_PER_CORE, F=2048, bufs=3, in_bufs=4, warmup_mms=12,
             dual_queue=True, debug=False):
    import concourse.bacc as bacc
    import concourse.mybir as mybir
    import concourse.tile as tile

    assert M % (P * F) == 0 and F % CH == 0
    # Taper the first/last tiles: shorter pipeline fill at the start and a
    # shorter compute drain after the last input DMA.
    taper = [F // 4] * 4
    body = (M // P - 2 * F) // F
    tiles = taper + [F] * body + taper
    assert sum(tiles) == M // P
    T = len(tiles)

    f32 = mybir.dt.float32
    i32 = mybir.dt.int32
    bf16 = mybir.dt.bfloat16
    Alu = mybir.AluOpType
    Act = mybir.ActivationFunctionType

    nc = bacc.Bacc("TRN2", target_bir_lowering=False, debug=debug,
                   num_devices=N_CORES)

    probs = nc.dram_tensor("probs", [M], f32, kind="ExternalInput")
    lbls = nc.dram_tensor("lbls", [M], i32, kind="ExternalInput")
    out_diag = nc.dram_tensor("out_diag", [2, P, 3 * CH], f32,
                              kind="ExternalOutput")
    out_ones = nc.dram_tensor("out_ones", [1, 512], f32,
                              kind="ExternalOutput")
    out_acc = nc.dram_tensor("out_acc", [P, T], f32,
                             kind="ExternalOutput")

    # per-tile DRAM views: tile t covers columns [off, off+Ft) of [P, M//P]
    p2 = probs.ap().rearrange("(p f) -> p f", p=P, f=M // P)
    l2 = lbls.ap().rearrange("(p f) -> p f", p=P, f=M // P)

    with tile.TileContext(nc) as tc, ExitStack() as ctx:
        pin = ctx.enter_context(tc.tile_pool(name="pin", bufs=in_bufs))
        lin = ctx.enter_context(tc.tile_pool(name="lin", bufs=in_bufs))
        lbpool = ctx.enter_context(tc.tile_pool(name="lbpool", bufs=bufs))
        cpool = ctx.enter_context(tc.tile_pool(name="cpool", bufs=bufs))
        jpool = ctx.enter_context(tc.tile_pool(name="jpool", bufs=2))
        accp = ctx.enter_context(tc.tile_pool(name="accp", bufs=1))
        psump = ctx.enter_context(tc.tile_pool(name="psump", bufs=1,
                                               space="PSUM"))

        # acc columns 0:T = Npos partials
        acc = accp.tile([P, T], f32)
        ones = accp.tile([P, 1], bf16)
        nc.vector.memset(ones[:], 1.0)

        # two alternating diag accumulators (even / odd chunks) so
        # back-to-back accumulating matmuls don't chain on one PSUM bank
        ps_diag0 = psump.tile([P, 3 * CH], f32)
        ps_diag1 = psump.tile([P, 3 * CH], f32)
        ps_y = psump.tile([1, 512], f32)
        # zero-fill so unwritten tail columns (sub-512 ones-MMs) read as 0
        nc.vector.memset(ps_y[:], 0.0)

        # Warm the PE HAM clock-gate (1.2 -> 2.4 GHz needs ~3.4us sustained)
        # while the first input DMAs are in flight.
        if warmup_mms:
            wu = accp.tile([P, 3 * CH], bf16)
            nc.vector.memset(wu[:], 0.0)
            ps_wu = psump.tile([P, 3 * CH], f32)
            for i in range(warmup_mms):
                nc.tensor.matmul(ps_wu[:, :], wu[:, :CH], wu[:],
                                 start=(i == 0), stop=(i == warmup_mms - 1))

        nctot = sum(tiles) // CH        # total diag chunks
        bank = [i % 2 for i in range(nctot)]
        b0_stop = max(i for i, b in enumerate(bank) if b == 0)
        b1_stop = max(i for i, b in enumerate(bank) if b == 1)
        ymms = sum(-(-(Ft // CH) // 4) for Ft in tiles)  # total ones-MMs
        ci = 0
        yi = 0
        off = 0
        diag_sb = accp.tile([P, 2 * 3 * CH], f32)
        for t, Ft in enumerate(tiles):
            NCt = Ft // CH
            sl = slice(off, off + Ft)
            off += Ft

            pt = pin.tile([P, F], f32, tag="pt")
            nc.sync.dma_start(out=pt[:, :Ft], in_=p2[:, sl])
            lt = lin.tile([P, F], i32, tag="lt")
            nc.sync.dma_start(out=lt[:, :Ft], in_=l2[:, sl])

            # lb = bf16(l); fused accum -> Npos partials
            lb = lbpool.tile([P, F], bf16, tag="lb")
            nc.scalar.activation(lb[:, :Ft], lt[:, :Ft], Act.Copy,
                                 accum_out=acc[:, t:t + 1])

            comb = cpool.tile([P, 3 * F], bf16, tag="comb")
            c4 = comb[:].rearrange("p (c s j) -> p c s j", c=F // CH, s=3,
                                   j=CH)[:, :NCt]
            pt4 = pt[:, :Ft].rearrange("p (c j) -> p c j", c=NCt, j=CH)

            # slot 0: pb = bf16(p)
            nc.vector.tensor_copy(out=c4[:, :, 0, :], in_=pt4)
            # slot 2: g = [p > 0.5]  (fp32 compare)
            nc.vector.tensor_scalar(out=c4[:, :, 2, :], in0=pt4,
                                    scalar1=0.5, scalar2=None, op0=Alu.is_gt)
            # slot 1: y = g * pb
            nc.vector.tensor_mul(out=c4[:, :, 1, :], in0=c4[:, :, 2, :],
                                 in1=c4[:, :, 0, :])

            # diag reductions: ps_diag{0,1} += lb_c.T @ comb_c
            for c in range(NCt):
                ps = ps_diag0 if bank[ci] == 0 else ps_diag1
                nc.tensor.matmul(
                    ps[:, :], lb[:, c * CH:(c + 1) * CH],
                    c4[:, c, :, :],
                    start=(ci in (0, 1)),
                    stop=(ci in (b0_stop, b1_stop)))
                ci += 1

            # Y = sum(y) via ones-matmuls (<=4 chunks of the y plane per MM)
            for c in range(0, NCt, 4):
                hi = min(c + 4, NCt)
                nc.tensor.matmul(
                    ps_y[0:1, :(hi - c) * CH], ones[:], c4[:, c:hi, 1, :],
                    start=(yi == 0), stop=(yi == ymms - 1))
                yi += 1

        # PSUM -> SBUF -> DRAM (bank0 on ACT in parallel with DVE copies)
        nc.scalar.copy(diag_sb[:, :3 * CH], ps_diag0[:, :])
        nc.vector.tensor_copy(out=diag_sb[:, 3 * CH:], in_=ps_diag1[:, :])
        ones_sb = accp.tile([1, 512], f32)
        nc.vector.tensor_copy(out=ones_sb[:], in_=ps_y[0:1, :])

        d3 = out_diag.ap().rearrange("b p c -> p b c")
        sb3 = diag_sb[:].rearrange("p (b c) -> p b c", b=2, c=3 * CH)
        nc.sync.dma_start(out=d3, in_=sb3)
        nc.sync.dma_start(out=out_ones.ap(), in_=ones_sb[:])
        nc.sync.dma_start(out=out_acc.ap(), in_=acc[:])

    nc.compile()
    return nc, T


def get_nc():
    if "nc" not in _CACHE:
        _CACHE["nc"] = build_nc()
    return _CACHE["nc"]


def run_device(probs, lbls, trace=False, **run_kwargs):
    """Run the SPMD kernel; returns (per-core result dicts, BassKernelResults)."""
    from concourse import bass_utils

    nc, _ = get_nc()
    probs = np.ascontiguousarray(probs, dtype=np.float32)
    lbls = np.ascontiguousarray(lbls, dtype=np.int32)
    assert probs.shape == (N_TOTAL,) and lbls.shape == (N_TOTAL,)
    M = M_PER_CORE
    in_maps = [
        {"probs": probs[c * M:(c + 1) * M], "lbls": lbls[c * M:(c + 1) * M]}
        for c in range(N_CORES)
    ]
    res = bass_utils.run_bass_kernel_spmd(
        nc, in_maps, core_ids=list(range(N_CORES)), trace=trace, **run_kwargs)
    return res.results, res


def combine(results):
    """Combine per-core partial sums into the final -f1 scalar."""
    Sx = TP = C = Y = Npos = 0.0
    for r in results:
        dg = np.asarray(r["out_diag"], dtype=np.float64).reshape(2, P, 3, CH)
        for b in range(2):
            Sx += np.trace(dg[b, :, 0, :])
            TP += np.trace(dg[b, :, 1, :])
            C += np.trace(dg[b, :, 2, :])
        Y += np.asarray(r["out_ones"], dtype=np.float64).sum()
        Npos += np.asarray(r["out_acc"], dtype=np.float64).sum()

    FP = Y - TP
    FN = Npos - C - Sx + TP
    precision = (TP + EPS) / (TP + FP + EPS)
    recall = (TP + EPS) / (TP + FN + EPS)
    f1 = 2.0 * precision * recall / (precision + recall)
    return np.float32(-f1)


def kernel(probs, lbls):
    results, _ = run_device(probs, lbls)
    return np.asarray(combine(results), dtype=np.float32)


if __name__ == "__main__":
    rng = np.random.default_rng(0)
    p = rng.uniform(0, 1, N_TOTAL).astype(np.float32)
    l = rng.integers(0, 2, N_TOTAL).astype(np.int32)
    out = kernel(p, l)
    print("kernel output:", out)
